# revision 3
# baseline (speedup 1.0000x reference)
"""Block-diagonal linear for Trainium2 (8 NeuronCores, batch-data-parallel).

y[b,c,o] = sum_i x[b,c,i]*W[c,o,i] + bias[c,o], x [16384, 3072] f32.
Sharding: batch split 8 ways (2048 rows/core); W/bias replicated.

TensorE formulation: Wbig [3072,3072] is block-diagonal at c-group (3x3)
granularity; tiled into 25 c-aligned diagonal blocks per 128-row x-tile
(23 blocks of 42 c-groups = 126 wide, 2 of 29 = 87 wide). Per block:
  xT_blk = transpose(x_tile[:, f0:f0+fw])     (TensorE, fp16 PSUM)
  y_blk  = matmul(lhsT=xT_blk, rhs=W_blk)     (TensorE, f32 PSUM)
y lands in natural [b, f] layout with no transpose-back. The bias rides
the spare contraction row: lhsT row 126 is a ones-row (GpSimd memset),
W row 126 carries the bias, so matmul adds bias for free and the
PSUM->SBUF drains are plain copies balanced across DVE (2x fp16 for xT)
and ACT.

Memory-regime key: device I/O is fp16 (the kernel computes in fp16
anyway). The host casts x f32->fp16 before upload and y fp16->f32 after
download, halving device HBM traffic to 25.2 MB/core. Plain HWDGE DMAs:
x in on the Sync ring, weights + y out on the ACT ring.
"""

import numpy as np

import concourse.bacc as bacc
import concourse.mybir as mybir
from concourse import bass_utils, masks
from concourse.tile import TileContext

N_CORES = 8
B_FULL = 16384
F = 3072
C = F // 3  # 1024
B_CORE = B_FULL // N_CORES  # 2048
P = 128
GROUPS = [1] * 16  # tiles per DMA group
FP32 = mybir.dt.float32
FP16 = mybir.dt.float16

# Diagonal blocks in c-group space: 23 x 42 + 2 x 29 = 1024.
BLOCK_NC = [42] * 23 + [29, 29]
BLOCK_C0 = np.cumsum([0] + BLOCK_NC).tolist()[:-1]
NBLK = len(BLOCK_NC)  # 25
XT_CHUNK = 7  # transpose blocks per PSUM bank chunk (7*128 fp16 = 1792B)
YW_MAX = 504  # f32 cols per y PSUM chunk (<= 512 = one bank)
ONES_ROW = 126  # spare contraction row carrying the bias ones


def _blocks():
    out = []
    for k in range(NBLK):
        c0, ncg = BLOCK_C0[k], BLOCK_NC[k]
        out.append((3 * c0, 3 * ncg))  # (f0, fw)
    return out


def _y_chunks():
    """Pack consecutive blocks into <=YW_MAX-wide f32 PSUM chunks."""
    chunks, cur = [], []
    w = 0
    for k, (f0, fw) in enumerate(_blocks()):
        if w + fw > YW_MAX:
            chunks.append(cur)
            cur, w = [], 0
        cur.append(k)
        w += fw
    chunks.append(cur)
    return chunks


def build_bass():
    nc = bacc.Bacc("TRN2", num_devices=N_CORES)
    x = nc.dram_tensor("xh", [B_CORE, F], FP16, kind="ExternalInput")
    wsb = nc.dram_tensor("wsb", [P, NBLK * P], FP16, kind="ExternalInput")
    y = nc.dram_tensor("yh", [B_CORE, F], FP16, kind="ExternalOutput")

    blocks = _blocks()
    ychunks = _y_chunks()

    with TileContext(nc) as tc:
        with (
            tc.tile_pool(name="wpool", bufs=1) as wpool,
            tc.tile_pool(name="xpool", bufs=6) as xpool,
            tc.tile_pool(name="ypool", bufs=4) as ypool,
            tc.tile_pool(name="xtsb", bufs=6) as xtsb_pool,
            tc.tile_pool(name="xtps", bufs=4, space="PSUM") as xtps_pool,
            tc.tile_pool(name="yps", bufs=4, space="PSUM") as yps_pool,
        ):
            wsb_sb = wpool.tile([P, NBLK * P], FP16)
            ident = wpool.tile([P, P], FP16)
            # weights on the ACT ring so the Sync ring starts with x tile 0
            nc.scalar.dma_start(out=wsb_sb[:, :], in_=wsb.ap()[:, :])
            masks.make_identity(nc, ident[:, :])

            tile0 = 0
            for g, gt in enumerate(GROUPS):
                r0 = tile0 * P
                tile0 += gt
                x16 = xpool.tile([P, gt * F], FP16, tag="x", name=f"x16_{g}")
                y16 = ypool.tile([P, gt * F], FP16, tag="y", name=f"y16_{g}")
                xdram = x.ap()[r0 : r0 + gt * P, :].rearrange(
                    "(t p) f -> p t f", p=P
                )
                ydram = y.ap()[r0 : r0 + gt * P, :].rearrange(
                    "(t p) f -> p t f", p=P
                )
                # plain fp16 DMA in (HWDGE, Sync ring); first tile split by
                # xT-chunk column range so its transposes start sooner
                xsb3 = x16[:, :].rearrange("p (t f) -> p t f", f=F)
                if g == 0:
                    csplits = [0, 896, 1792, 2688, F]
                    for a, b_ in zip(csplits, csplits[1:]):
                        nc.sync.dma_start(
                            out=xsb3[:, :, a:b_], in_=xdram[:, :, a:b_]
                        )
                else:
                    nc.sync.dma_start(out=xsb3, in_=xdram)
                x4 = x16[:, :].rearrange("p (t f) -> p t f", f=F)
                y4 = y16[:, :].rearrange("p (t f) -> p t f", f=F)

                for tl in range(gt):
                    # --- transpose all blocks, chunked into PSUM banks ---
                    xt_sb = {}  # block idx -> (sbuf tile, slot col)
                    for ci in range(0, NBLK, XT_CHUNK):
                        bs = range(ci, min(ci + XT_CHUNK, NBLK))
                        xt_ps = xtps_pool.tile(
                            [P, XT_CHUNK * P], FP16, tag="xtps"
                        )
                        sb = xtsb_pool.tile(
                            [P, XT_CHUNK * P], FP16, tag="xtsb"
                        )
                        twds = []
                        for j, k in enumerate(bs):
                            f0, fw = blocks[k]
                            twd = min(P, F - f0)  # pad width (reads
                            # into the next block's columns; rows fw..twd
                            # of the slot are junk and never read back)
                            nc.tensor.transpose(
                                xt_ps[0:twd, j * P : j * P + P],
                                x4[:, tl, f0 : f0 + twd],
                                ident[:, :],
                            )
                            twds.append(twd)
                            xt_sb[k] = (sb, j * P)
                        ncols = len(bs) * P
                        # Pre-fill rows 64..128 with 1.0 (32-aligned partition
                        # base). The data copy below overwrites rows 0:126, so
                        # row 126 stays 1.0 = the bias ones-row; leftover 1.0s
                        # sit only in rows that multiply zero weight rows.
                        nc.gpsimd.memset(sb[64:P, 0:ncols], 1.0)
                        # copy written regions below the ones-row; all slots
                        # are full-height except the last block's
                        n128 = sum(1 for t in twds if t == P)
                        if n128:
                            nc.vector.tensor_copy(
                                out=sb[0:ONES_ROW, 0 : n128 * P],
                                in_=xt_ps[0:ONES_ROW, 0 : n128 * P],
                            )
                        for j in range(n128, len(twds)):
                            nc.vector.tensor_copy(
                                out=sb[0 : twds[j], j * P : j * P + P],
                                in_=xt_ps[0 : twds[j], j * P : j * P + P],
                            )
                    # --- block-diagonal matmuls (bias via ones-row) ---
                    for m, yc in enumerate(ychunks):
                        yf0 = blocks[yc[0]][0]
                        ycw = sum(blocks[k][1] for k in yc)
                        y_ps = yps_pool.tile([P, YW_MAX], FP32, tag="yps")
                        for k in yc:
                            f0, fw = blocks[k]
                            sb, col = xt_sb[k]
                            nc.tensor.matmul(
                                y_ps[:, f0 - yf0 : f0 - yf0 + fw],
                                sb[0 : ONES_ROW + 1, col : col + P],
                                wsb_sb[0 : ONES_ROW + 1, k * P : k * P + fw],
                                start=True,
                                stop=True,
                            )
                        # PSUM -> SBUF drain, split DVE / ACT
                        if m < 2:
                            nc.vector.tensor_copy(
                                out=y4[:, tl, yf0 : yf0 + ycw],
                                in_=y_ps[:, 0:ycw],
                            )
                        else:
                            nc.scalar.copy(
                                y4[:, tl, yf0 : yf0 + ycw], y_ps[:, 0:ycw]
                            )

                # plain fp16 DMA out (HWDGE, ACT ring); last tile split so
                # the flush starts before the final copies finish
                ysb3 = y16[:, :].rearrange("p (t f) -> p t f", f=F)
                if g == len(GROUPS) - 1:
                    for a, b_ in [(0, 1512), (1512, F)]:
                        nc.scalar.dma_start(
                            out=ydram[:, :, a:b_], in_=ysb3[:, :, a:b_]
                        )
                else:
                    nc.scalar.dma_start(out=ydram, in_=ysb3)

    nc.compile()
    return nc


def _prep_small(W, b):
    """Host-side weight image (fp16): diagonal blocks + bias ones-row."""
    wimg = np.zeros((P, NBLK * P), dtype=np.float16)
    bflat = b.reshape(F).astype(np.float16)
    for k in range(NBLK):
        c0, ncg = BLOCK_C0[k], BLOCK_NC[k]
        f0, fw = 3 * c0, 3 * ncg
        blk = np.zeros((ncg, 3, ncg, 3), dtype=np.float32)
        idx = np.arange(ncg)
        # Wblock[3u+i, 3u+o] = W[c0+u, o, i]
        blk[idx, :, idx, :] = W[c0 : c0 + ncg].transpose(0, 2, 1)
        wimg[0:fw, k * P : k * P + fw] = blk.reshape(fw, fw).astype(np.float16)
        wimg[ONES_ROW, k * P : k * P + fw] = bflat[f0 : f0 + fw]
    return wimg


def run(x, W, b, trace=False, **run_kwargs):
    nc = build_bass()
    wimg = _prep_small(np.asarray(W), np.asarray(b))
    xh = np.asarray(x).astype(np.float16)
    in_maps = [
        {
            "xh": np.ascontiguousarray(xh[k * B_CORE : (k + 1) * B_CORE]),
            "wsb": wimg,
        }
        for k in range(N_CORES)
    ]
    res = bass_utils.run_bass_kernel_spmd(
        nc, in_maps, core_ids=list(range(N_CORES)), trace=trace, **run_kwargs
    )
    y = np.concatenate(
        [r["yh"] for r in res.results], axis=0
    ).astype(np.float32)
    return y, res


def kernel(x, W, b):
    y, _ = run(x, W, b, trace=False)
    return y
